# revision 2
# baseline (speedup 1.0000x reference)
"""Trainium2 Bass kernel for a 2-layer Chebyshev GCN (K=3) over a random graph.

Contract: kernel(**inputs) takes the FULL unsharded inputs (as produced by the
problem's setup_inputs) and returns the FULL output [N, out_f] float32.

Strategy (8 NeuronCores, SPMD single NEFF):
  - Nodes sharded contiguously: core c owns rows [c*RPC, (c+1)*RPC).
  - Edges sharded by destination row; per core sorted by local row, grouped
    into 128-row blocks, packed into a UNIFORM number (CPB) of 128-edge
    chunks per block so every block runs the identical program.
  - All per-block work runs inside tc.For_i hardware loops (manual unroll 2)
    to keep the instruction count (and hence per-call walrus-compile time,
    which dominates the measured wall) small.
  - propagate(T)[r] = -dis[r] * sum_{e: row=r} w_e * (dis*T)[col_e]:
      * scaled table Ts = dis*T replicated in DRAM (bf16) via AllGather;
      * per chunk, 128 source rows fetched with a [128,1]-offset indirect
        DMA gather (index column staged to a fixed SBUF address first —
        the HW requires a physical AP for the indirect offset);
      * segment-sum as one-hot matmul accumulated in PSUM over the chunks.
  - Degrees are computed on device from the shipped edge weights (one-hot
    matmul against a ones-free rhs), saving the wdeg input tensor.
  - Dense phases (X @ W, BatchNorm, final linear) per 128-row tile with PE
    transposes; BN batch stats accumulated in SBUF inside the dense loop.
"""

import sys

import numpy as np

sys.path.insert(0, "/opt/trn_rl_repo")

import ml_dtypes

BF16 = ml_dtypes.bfloat16

# ---------------------------------------------------------------------------
# Host-side preprocessing: shard + sort + pack edges, build per-core inputs.
# ---------------------------------------------------------------------------


class Meta:
    pass


def _host_prep(x, edge_index, edge_weight, W1, b1, W2, b2, bn_gamma, bn_beta,
               lin_W, lin_b, n_cores=8):
    m = Meta()
    N, in_f = x.shape
    E = edge_index.shape[1]
    m.N, m.E, m.C = int(N), int(E), int(n_cores)
    m.in_f = int(in_f)
    m.c1 = int(W1.shape[2])
    m.c2 = int(W2.shape[2])
    m.out_f = int(lin_W.shape[0])
    assert N % n_cores == 0
    m.RPC = N // n_cores                      # real rows per core
    m.NB = (m.RPC + 127) // 128               # 128-row blocks per core
    m.NP = m.NB * 128                         # padded rows per core
    m.TN = m.C * m.NP                         # replicated table rows
    m.F = max(m.in_f, m.c1, m.c2)             # widest feature dim (64)

    row = np.asarray(edge_index[0], dtype=np.int64)
    col = np.asarray(edge_index[1], dtype=np.int64)
    w = np.asarray(edge_weight, dtype=np.float32)

    core = row // m.RPC
    lr = row - core * m.RPC                   # local row on owning core
    tcol = (col // m.RPC) * m.NP + (col % m.RPC)  # table coordinate of source

    order = np.lexsort((lr, core))
    core_s, lr_s, tcol_s, w_s = core[order], lr[order], tcol[order], w[order]
    bounds = np.searchsorted(core_s, np.arange(m.C + 1))

    # uniform chunks-per-block across all cores and blocks
    per_core = []
    bmax = 1
    for c in range(m.C):
        s, e = bounds[c], bounds[c + 1]
        lrc, tc, wc = lr_s[s:e], tcol_s[s:e], w_s[s:e]
        blk = lrc // 128
        bcount = np.bincount(blk, minlength=m.NB)
        bmax = max(bmax, int(bcount.max()) if len(lrc) else 1)
        per_core.append((lrc, tc, wc, blk, bcount))
    m.CPB = max((bmax + 127) // 128, 1)       # uniform chunks per block
    m.CH = m.NB * m.CPB                       # chunks per core

    in_maps = []
    shared = _shared_consts(m, W1, b1, W2, b2, bn_gamma, bn_beta, lin_W, lin_b)
    for c in range(m.C):
        lrc, tc, wc, blk, bcount = per_core[c]
        nloc = len(lrc)

        bstart = np.concatenate(([0], np.cumsum(bcount)))[:-1]
        within_blk = np.arange(nloc) - bstart[blk]
        slot = (blk * m.CPB) * 128 + within_blk    # flat chunk-slot index

        col_arr = np.zeros(m.CH * 128, dtype=np.int32)
        w_arr = np.zeros(m.CH * 128, dtype=np.uint8)
        d_arr = np.zeros(m.CH * 128, dtype=np.uint8)
        col_arr[slot] = tc
        # weights quantized to u8: the uniform scale cancels exactly in the
        # symmetric normalization (deg scales by s, dis by 1/sqrt(s))
        w_arr[slot] = np.clip(np.rint(wc * 255.0), 0, 255).astype(np.uint8)
        d_arr[slot] = (lrc % 128).astype(np.uint8)

        def to_sb(a):                         # [CH*128] -> [128, CH]
            return np.ascontiguousarray(a.reshape(m.CH, 128).T)

        xp = np.zeros((m.NP, m.in_f), dtype=np.float32)
        xp[:m.RPC] = np.asarray(x[c * m.RPC:(c + 1) * m.RPC], dtype=np.float32)

        im = dict(shared)
        im["xs"] = xp.astype(BF16)
        im["collo"] = to_sb(col_arr & 0xFFFF).astype(np.uint16)
        im["colhi"] = to_sb(col_arr >> 16).astype(np.uint8)
        im["wsb"] = to_sb(w_arr)
        im["dsb"] = to_sb(d_arr)
        in_maps.append(im)
    return m, in_maps


def _shared_consts(m, W1, b1, W2, b2, bn_gamma, bn_beta, lin_W, lin_b):
    W1 = np.asarray(W1, np.float32)
    W2 = np.asarray(W2, np.float32)
    sh = {}
    for k in range(3):
        sh[f"w1_{k}"] = W1[k].astype(BF16)
        sh[f"w2_{k}"] = W2[k].astype(BF16)
    sh["linwt"] = np.ascontiguousarray(np.asarray(lin_W, np.float32).T).astype(BF16)
    sh["b1rep"] = np.tile(np.asarray(b1, np.float32)[None, :], (128, 1))
    sh["b2rep"] = np.tile(np.asarray(b2, np.float32)[None, :], (128, 1))
    sh["linbrep"] = np.tile(np.asarray(lin_b, np.float32)[None, :], (128, 1))
    sh["gammarow"] = np.asarray(bn_gamma, np.float32)[None, :].copy()
    sh["betarow"] = np.asarray(bn_beta, np.float32)[None, :].copy()
    sh["id128"] = np.eye(128, dtype=np.float32).astype(BF16)
    sh["iotarep"] = np.tile(
        np.arange(128, dtype=np.float32).astype(BF16)[None, :], (128, 1))
    sh["onesrow"] = np.ones((1, 128), dtype=np.float32).astype(BF16)
    sh["onescol"] = np.ones((128, 1), dtype=np.float32)
    sh["piota"] = np.arange(128, dtype=np.float32)[:, None].copy()
    return sh


# ---------------------------------------------------------------------------
# Device program
# ---------------------------------------------------------------------------


def _build_program(m):
    import concourse.bass as bass
    import concourse.tile as tile
    from concourse import bacc, mybir

    f32 = mybir.dt.float32
    bf16 = mybir.dt.bfloat16
    i32 = mybir.dt.int32
    u8 = mybir.dt.uint8

    nc = bacc.Bacc(num_devices=m.C, num_swdge_queues=4)

    u16 = mybir.dt.uint16
    xs = nc.dram_tensor("xs", [m.NP, m.in_f], bf16, kind="ExternalInput")
    collo = nc.dram_tensor("collo", [128, m.CH], u16, kind="ExternalInput")
    colhi = nc.dram_tensor("colhi", [128, m.CH], u8, kind="ExternalInput")
    wsb = nc.dram_tensor("wsb", [128, m.CH], u8, kind="ExternalInput")
    dsb = nc.dram_tensor("dsb", [128, m.CH], u8, kind="ExternalInput")
    w1 = [nc.dram_tensor(f"w1_{k}", [m.in_f, m.c1], bf16, kind="ExternalInput")
          for k in range(3)]
    w2 = [nc.dram_tensor(f"w2_{k}", [m.c1, m.c2], bf16, kind="ExternalInput")
          for k in range(3)]
    linwt = nc.dram_tensor("linwt", [m.c2, m.out_f], bf16, kind="ExternalInput")
    b1rep = nc.dram_tensor("b1rep", [128, m.c1], f32, kind="ExternalInput")
    b2rep = nc.dram_tensor("b2rep", [128, m.c2], f32, kind="ExternalInput")
    linbrep = nc.dram_tensor("linbrep", [128, m.out_f], f32, kind="ExternalInput")
    gammarow = nc.dram_tensor("gammarow", [1, m.c1], f32, kind="ExternalInput")
    betarow = nc.dram_tensor("betarow", [1, m.c1], f32, kind="ExternalInput")
    id128 = nc.dram_tensor("id128", [128, 128], bf16, kind="ExternalInput")
    iotarep = nc.dram_tensor("iotarep", [128, 128], bf16, kind="ExternalInput")
    onesrow = nc.dram_tensor("onesrow", [1, 128], bf16, kind="ExternalInput")
    onescol = nc.dram_tensor("onescol", [128, 1], f32, kind="ExternalInput")
    piota = nc.dram_tensor("piota", [128, 1], f32, kind="ExternalInput")
    out = nc.dram_tensor("out", [m.NP, m.out_f], bf16, kind="ExternalOutput")

    T = dict(locals())
    for k in range(3):
        T[f"w1_{k}"] = w1[k]
        T[f"w2_{k}"] = w2[k]

    with tile.TileContext(nc) as tc:
        _emit(nc, tc, m, T)
    nc.finalize()
    return nc


def _emit(nc, tc, m, T):
    from contextlib import ExitStack

    import concourse.bass as bass
    from concourse import mybir
    from concourse.bass import ds, ts

    f32 = mybir.dt.float32
    bf16 = mybir.dt.bfloat16
    i32 = mybir.dt.int32
    OP = mybir.AluOpType
    rg = [list(range(m.C))]
    NB, CPB, F = m.NB, m.CPB, m.F

    with ExitStack() as ctx:
        cp = ctx.enter_context(tc.tile_pool(name="consts", bufs=1))
        bigp = ctx.enter_context(tc.tile_pool(name="big", bufs=4))
        stgp = ctx.enter_context(tc.tile_pool(name="stage", bufs=1))
        gp = ctx.enter_context(tc.tile_pool(name="gth", bufs=4))
        ep = ctx.enter_context(tc.tile_pool(name="epi", bufs=4))
        pp = ctx.enter_context(tc.tile_pool(name="ps", bufs=2, space="PSUM"))
        dp = ctx.enter_context(tc.tile_pool(name="dram", bufs=1, space="DRAM"))

        def load_const(name, shape, dtype):
            t = cp.tile(shape, dtype, tag=name, name=name)
            nc.sync.dma_start(out=t[:], in_=T[name][:])
            return t

        iota_s = load_const("iotarep", [128, 128], bf16)
        id_s = load_const("id128", [128, 128], bf16)
        onesrow_s = load_const("onesrow", [1, 128], bf16)
        onescol_s = load_const("onescol", [128, 1], f32)
        w1_s = [load_const(f"w1_{k}", [m.in_f, m.c1], bf16) for k in range(3)]
        w2_s = [load_const(f"w2_{k}", [m.c1, m.c2], bf16) for k in range(3)]
        linwt_s = load_const("linwt", [m.c2, m.out_f], bf16)
        b1r_s = load_const("b1rep", [128, m.c1], f32)
        b2r_s = load_const("b2rep", [128, m.c2], f32)
        linbr_s = load_const("linbrep", [128, m.out_f], f32)
        gam_s = load_const("gammarow", [1, m.c1], f32)
        bet_s = load_const("betarow", [1, m.c1], f32)

        # unpack compact edge tensors: col (u16+u8 -> i32, exact via f32),
        # w (u8 -> bf16), d (u8 -> bf16)
        col_s = cp.tile([128, m.CH], i32, tag="col_s", name="col_s")
        w_s = cp.tile([128, m.CH], bf16, tag="w_s", name="w_s")
        d_s = cp.tile([128, m.CH], bf16, tag="d_s", name="d_s")
        piota_s = load_const("piota", [128, 1], f32)
        rowmask_s = cp.tile([128, NB], f32, tag="rowmask", name="rowmask")
        nc.vector.memset(rowmask_s[:], 1.0)
        lastvalid = m.RPC - (NB - 1) * 128
        if lastvalid < 128:
            nc.vector.tensor_scalar(out=rowmask_s[:, NB - 1:NB],
                                    in0=piota_s[:], scalar1=float(lastvalid),
                                    scalar2=None, op0=OP.is_lt)
        with tc.tile_pool(name="unpack", bufs=1) as up:
            lo_u = up.tile([128, m.CH], mybir.dt.uint16, tag="lo_u",
                           name="lo_u")
            nc.sync.dma_start(out=lo_u[:], in_=T["collo"][:])
            hi_u = up.tile([128, m.CH], mybir.dt.uint8, tag="hi_u",
                           name="hi_u")
            nc.sync.dma_start(out=hi_u[:], in_=T["colhi"][:])
            wu8 = up.tile([128, m.CH], mybir.dt.uint8, tag="wu8", name="wu8")
            nc.sync.dma_start(out=wu8[:], in_=T["wsb"][:])
            du8 = up.tile([128, m.CH], mybir.dt.uint8, tag="du8", name="du8")
            nc.sync.dma_start(out=du8[:], in_=T["dsb"][:])
            lo_f = up.tile([128, m.CH], f32, tag="lo_f", name="lo_f")
            nc.vector.tensor_copy(out=lo_f[:], in_=lo_u[:])
            hi_f = up.tile([128, m.CH], f32, tag="hi_f", name="hi_f")
            nc.vector.tensor_copy(out=hi_f[:], in_=hi_u[:])
            nc.vector.tensor_scalar(out=hi_f[:], in0=hi_f[:],
                                    scalar1=65536.0, scalar2=None,
                                    op0=OP.mult)
            nc.vector.tensor_tensor(out=lo_f[:], in0=lo_f[:], in1=hi_f[:],
                                    op=OP.add)
            nc.vector.tensor_copy(out=col_s[:], in_=lo_f[:])
            nc.vector.tensor_copy(out=w_s[:], in_=wu8[:])
            nc.vector.tensor_copy(out=d_s[:], in_=du8[:])

        # ------------ degree via one-hot matmul, in a HW loop ------------
        deg = cp.tile([128, NB], f32, tag="deg", name="deg")

        def deg_body(i):
            wstage = gp.tile([128, CPB], bf16, tag="wstage", name="wstage")
            nc.vector.tensor_copy(out=wstage[:], in_=w_s[:, ts(i, CPB)])
            o = gp.tile([128, CPB * 128], bf16, tag="o", name="o")
            nc.vector.tensor_tensor(
                out=o[:].rearrange("p (c k) -> p c k", c=CPB),
                in0=iota_s[:].unsqueeze(1).broadcast_to([128, CPB, 128]),
                in1=d_s[:, ts(i, CPB)].unsqueeze(2)
                    .broadcast_to([128, CPB, 128]),
                op=OP.is_equal)
            dps = pp.tile([128, F], f32, tag="prop", name="prop")
            for j in range(CPB):
                nc.tensor.matmul(out=dps[:, 0:1],
                                 lhsT=o[:, j * 128:(j + 1) * 128],
                                 rhs=wstage[:, j:j + 1],
                                 start=(j == 0), stop=(j == CPB - 1))
            nc.vector.tensor_copy(out=deg[:, ds(i, 1)], in_=dps[:, 0:1])

        with tc.For_i(0, NB, 2) as i:
            deg_body(i)
            deg_body(i + 1)

        # ------------ deg -> dis vectors ------------
        def cvec(tag):
            return cp.tile([128, NB], f32, tag=tag, name=tag)

        negmask = cvec("negmask")
        degsafe = cvec("degsafe")
        rinv = cvec("rinv")
        rs = cvec("rs")
        dis = cvec("dis")
        negdis = cvec("negdis")
        negdis2 = cvec("negdis2")
        negdisx2 = cvec("negdisx2")
        nc.vector.tensor_scalar(out=negmask[:], in0=deg[:], scalar1=0.0,
                                scalar2=-1.0, op0=OP.is_gt, op1=OP.mult)
        nc.vector.tensor_scalar(out=degsafe[:], in0=deg[:], scalar1=1e-20,
                                scalar2=None, op0=OP.max)
        nc.vector.reciprocal(out=rinv[:], in_=degsafe[:])
        nc.scalar.sqrt(out=rs[:], in_=rinv[:])
        nc.vector.tensor_scalar(out=dis[:], in0=rs[:], scalar1=-1.0,
                                scalar2=None, op0=OP.mult)
        nc.vector.tensor_tensor(out=dis[:], in0=dis[:], in1=negmask[:],
                                op=OP.mult)
        nc.vector.tensor_tensor(out=negdis[:], in0=rs[:], in1=negmask[:],
                                op=OP.mult)
        nc.vector.tensor_tensor(out=negdis2[:], in0=rinv[:], in1=negmask[:],
                                op=OP.mult)
        nc.vector.tensor_scalar(out=negdisx2[:], in0=negdis[:], scalar1=2.0,
                                scalar2=None, op0=OP.mult)

        # ------------ big persistent activations ------------
        def bigtile(f):
            return bigp.tile([128, NB * f], f32, tag="big", name="big")

        x_sb = bigtile(F)
        nc.gpsimd.dma_start(
            out=x_sb[:, :NB * m.in_f].rearrange("p (b f) -> p b f", b=NB),
            in_=T["xs"][:].rearrange("(b p) f -> p b f", p=128))

        stage = stgp.tile([128, NB * F], bf16, tag="stage", name="stage")

        sh = [dp.tile([m.NP, m.in_f], bf16, tag="sh0", name="sh0"),
              dp.tile([m.NP, m.in_f], bf16, tag="sh1", name="sh1"),
              dp.tile([m.NP, m.c1], bf16, tag="sh2", name="sh2"),
              dp.tile([m.NP, m.c1], bf16, tag="sh3", name="sh3")]
        tb = [dp.tile([m.TN, m.in_f], bf16, tag="tb0", name="tb0",
                      addr_space="Shared"),
              dp.tile([m.TN, m.in_f], bf16, tag="tb1", name="tb1",
                      addr_space="Shared"),
              dp.tile([m.TN, m.c1], bf16, tag="tb2", name="tb2",
                      addr_space="Shared"),
              dp.tile([m.TN, m.c1], bf16, tag="tb3", name="tb3",
                      addr_space="Shared")]

        def stage_to_table(i, f):
            nc.sync.dma_start(
                out=sh[i][:].rearrange("(b p) f -> p b f", p=128),
                in_=stage[:, :NB * f].rearrange("p (b f) -> p b f", b=NB))
            nc.gpsimd.collective_compute(
                "AllGather", OP.bypass, replica_groups=rg,
                ins=[sh[i][:]], outs=[tb[i][:]])

        # table0 = dis * x  (single broadcast op over all blocks)
        nc.vector.tensor_tensor(
            out=stage[:, :NB * m.in_f].rearrange("p (b f) -> p b f", b=NB),
            in0=x_sb[:, :NB * m.in_f].rearrange("p (b f) -> p b f", b=NB),
            in1=dis[:].unsqueeze(2).broadcast_to([128, NB, m.in_f]),
            op=OP.mult)
        stage_to_table(0, m.in_f)

        # ------------ the propagate primitive (HW loop) ------------
        qctr = [0]

        def propagate(table, f, handler):
            def body(i):
                colstage = gp.tile([128, CPB], i32, tag="colstage",
                                   name="colstage")
                nc.vector.tensor_copy(out=colstage[:],
                                      in_=col_s[:, ts(i, CPB)])
                g = gp.tile([128, CPB * f], bf16, tag="g", name="g")
                for j in range(CPB):
                    inst = nc.gpsimd.indirect_dma_start(
                        out=g[:, j * f:(j + 1) * f], out_offset=None,
                        in_=table[:],
                        in_offset=bass.IndirectOffsetOnAxis(
                            ap=colstage[:, j:j + 1], axis=0))
                    qn = qctr[0] % 4
                    qctr[0] += 1
                    if qn:
                        inst.ins.queue = f"qPoolDynamic{qn}"
                gw = gp.tile([128, CPB * f], bf16, tag="gw", name="gw")
                nc.vector.tensor_tensor(
                    out=gw[:].rearrange("p (c f) -> p c f", c=CPB),
                    in0=g[:].rearrange("p (c f) -> p c f", c=CPB),
                    in1=w_s[:, ts(i, CPB)].unsqueeze(2)
                        .broadcast_to([128, CPB, f]),
                    op=OP.mult)
                o = gp.tile([128, CPB * 128], bf16, tag="o", name="o")
                nc.vector.tensor_tensor(
                    out=o[:].rearrange("p (c k) -> p c k", c=CPB),
                    in0=iota_s[:].unsqueeze(1).broadcast_to([128, CPB, 128]),
                    in1=d_s[:, ts(i, CPB)].unsqueeze(2)
                        .broadcast_to([128, CPB, 128]),
                    op=OP.is_equal)
                psum = pp.tile([128, F], f32, tag="prop", name="prop")
                for j in range(CPB):
                    nc.tensor.matmul(
                        out=psum[:, :f],
                        lhsT=o[:, j * 128:(j + 1) * 128],
                        rhs=gw[:, j * f:(j + 1) * f],
                        start=(j == 0), stop=(j == CPB - 1))
                handler(i, psum[:, :f])

            with tc.For_i(0, NB, 2) as i:
                body(i)
                body(i + 1)

        # ------------ conv1 ------------
        T1 = bigtile(F)

        def h1_prop1(i, ps):
            nc.vector.tensor_scalar(
                out=T1[:, ts(i, m.in_f)], in0=ps,
                scalar1=negdis[:, ds(i, 1)], scalar2=None, op0=OP.mult)
            nc.vector.tensor_scalar(
                out=stage[:, ts(i, m.in_f)], in0=ps,
                scalar1=negdis2[:, ds(i, 1)], scalar2=None, op0=OP.mult)

        propagate(tb[0][:], m.in_f, h1_prop1)
        stage_to_table(1, m.in_f)

        T2 = bigtile(F)

        def h1_prop2(i, ps):
            t = ep.tile([128, F], f32, tag="tmp", name="tmp")
            nc.vector.tensor_scalar(
                out=t[:, :m.in_f], in0=ps,
                scalar1=negdisx2[:, ds(i, 1)], scalar2=None, op0=OP.mult)
            nc.vector.tensor_tensor(
                out=T2[:, ts(i, m.in_f)], in0=t[:, :m.in_f],
                in1=x_sb[:, ts(i, m.in_f)], op=OP.subtract)

        propagate(tb[1][:], m.in_f, h1_prop2)

        # dense conv1 + BN batch-stat accumulation
        h_sb = bigtile(F)
        s1acc = cp.tile([1, m.c1], f32, tag="s1acc", name="s1acc")
        s2acc = cp.tile([1, m.c1], f32, tag="s2acc", name="s2acc")
        nc.vector.memset(s1acc[:], 0.0)
        nc.vector.memset(s2acc[:], 0.0)

        def dense3(srcs, ws, fin, fout, i):
            hp = pp.tile([128, F], f32, tag="dense", name="dense")
            for k in range(3):
                cb = ep.tile([128, F], bf16, tag="cast", name="cast")
                nc.vector.tensor_copy(out=cb[:, :fin],
                                      in_=srcs[k][:, ts(i, fin)])
                tp = pp.tile([F, 128], bf16, tag="tp", name="tp")
                nc.tensor.transpose(out=tp[:fin, :], in_=cb[:, :fin],
                                    identity=id_s[:])
                tT = ep.tile([F, 128], bf16, tag="tT", name="tT")
                nc.scalar.copy(out=tT[:fin, :], in_=tp[:fin, :])
                nc.tensor.matmul(out=hp[:, :fout], lhsT=tT[:fin, :],
                                 rhs=ws[k][:], start=(k == 0), stop=(k == 2))
            return hp

        def dense1_body(i):
            hp = dense3([x_sb, T1, T2], w1_s, m.in_f, m.c1, i)
            hblk = ep.tile([128, m.c1], f32, tag="hblk", name="hblk")
            nc.vector.tensor_tensor(out=hblk[:], in0=hp[:, :m.c1],
                                    in1=b1r_s[:], op=OP.add)
            nc.vector.tensor_scalar(out=hblk[:], in0=hblk[:], scalar1=0.0,
                                    scalar2=None, op0=OP.max)
            nc.vector.tensor_scalar(out=hblk[:], in0=hblk[:],
                                    scalar1=rowmask_s[:, ds(i, 1)],
                                    scalar2=None, op0=OP.mult)
            nc.vector.tensor_copy(out=h_sb[:, ts(i, m.c1)], in_=hblk[:])
            hsq = ep.tile([128, m.c1], f32, tag="sq", name="sq")
            nc.scalar.square(out=hsq[:], in_=hblk[:])
            ps1 = pp.tile([1, m.c1], f32, tag="stats", name="stats")
            nc.tensor.matmul(out=ps1[:], lhsT=onescol_s[:], rhs=hblk[:],
                             start=True, stop=True)
            nc.vector.tensor_tensor(out=s1acc[:], in0=s1acc[:], in1=ps1[:],
                                    op=OP.add)
            ps2 = pp.tile([1, m.c1], f32, tag="stats", name="stats")
            nc.tensor.matmul(out=ps2[:], lhsT=onescol_s[:], rhs=hsq[:],
                             start=True, stop=True)
            nc.vector.tensor_tensor(out=s2acc[:], in0=s2acc[:], in1=ps2[:],
                                    op=OP.add)

        with tc.For_i(0, NB, 2) as i:
            dense1_body(i)
            dense1_body(i + 1)

        # ------------ BatchNorm (global batch stats) ------------
        stats_sb = cp.tile([1, 2 * m.c1], f32, tag="stats_sb", name="stats_sb")
        nc.vector.tensor_copy(out=stats_sb[:, :m.c1], in_=s1acc[:])
        nc.vector.tensor_copy(out=stats_sb[:, m.c1:], in_=s2acc[:])
        st_l = dp.tile([1, 2 * m.c1], f32, tag="st_l", name="st_l")
        st_g = dp.tile([1, 2 * m.c1], f32, tag="st_g", name="st_g",
                       addr_space="Shared")
        nc.sync.dma_start(out=st_l[:], in_=stats_sb[:])
        nc.gpsimd.collective_compute("AllReduce", OP.add, replica_groups=rg,
                                     ins=[st_l[:]], outs=[st_g[:]])
        gst = cp.tile([1, 2 * m.c1], f32, tag="gst", name="gst")
        nc.sync.dma_start(out=gst[:], in_=st_g[:])

        def row(tag):
            return cp.tile([1, m.c1], f32, tag=tag, name=tag)

        mu, ex2, var, vrec, vrs, gprow, bprow = (row(t) for t in
            ("mu", "ex2", "var", "vrec", "vrs", "gprow", "bprow"))
        inv_n = 1.0 / float(m.N)
        nc.vector.tensor_scalar(out=mu[:], in0=gst[:, :m.c1], scalar1=inv_n,
                                scalar2=None, op0=OP.mult)
        nc.vector.tensor_scalar(out=ex2[:], in0=gst[:, m.c1:], scalar1=inv_n,
                                scalar2=None, op0=OP.mult)
        nc.vector.tensor_tensor(out=var[:], in0=mu[:], in1=mu[:], op=OP.mult)
        nc.vector.tensor_tensor(out=var[:], in0=ex2[:], in1=var[:],
                                op=OP.subtract)
        nc.vector.tensor_scalar(out=var[:], in0=var[:], scalar1=1e-5,
                                scalar2=None, op0=OP.add)
        nc.vector.reciprocal(out=vrec[:], in_=var[:])
        nc.scalar.sqrt(out=vrs[:], in_=vrec[:])
        nc.vector.tensor_tensor(out=gprow[:], in0=gam_s[:], in1=vrs[:],
                                op=OP.mult)
        nc.vector.tensor_tensor(out=bprow[:], in0=mu[:], in1=gprow[:],
                                op=OP.mult)
        nc.vector.tensor_tensor(out=bprow[:], in0=bet_s[:], in1=bprow[:],
                                op=OP.subtract)
        gprow_bf = cp.tile([1, m.c1], bf16, tag="gprow_bf", name="gprow_bf")
        bprow_bf = cp.tile([1, m.c1], bf16, tag="bprow_bf", name="bprow_bf")
        nc.vector.tensor_copy(out=gprow_bf[:], in_=gprow[:])
        nc.vector.tensor_copy(out=bprow_bf[:], in_=bprow[:])
        grep = cp.tile([128, m.c1], f32, tag="grep", name="grep")
        brep = cp.tile([128, m.c1], f32, tag="brep", name="brep")
        for rowv, rep in ((gprow_bf, grep), (bprow_bf, brep)):
            rp = pp.tile([128, F], f32, tag="dense", name="dense")
            nc.tensor.matmul(out=rp[:, :m.c1], lhsT=onesrow_s[:],
                             rhs=rowv[:], start=True, stop=True)
            nc.scalar.copy(out=rep[:], in_=rp[:, :m.c1])

        # h' = g'*h + b' and table2 = dis*h' — broadcast ops over all blocks
        nc.vector.tensor_tensor(
            out=h_sb[:, :NB * m.c1].rearrange("p (b f) -> p b f", b=NB),
            in0=h_sb[:, :NB * m.c1].rearrange("p (b f) -> p b f", b=NB),
            in1=grep[:].unsqueeze(1).broadcast_to([128, NB, m.c1]),
            op=OP.mult)
        nc.vector.tensor_tensor(
            out=h_sb[:, :NB * m.c1].rearrange("p (b f) -> p b f", b=NB),
            in0=h_sb[:, :NB * m.c1].rearrange("p (b f) -> p b f", b=NB),
            in1=brep[:].unsqueeze(1).broadcast_to([128, NB, m.c1]),
            op=OP.add)
        nc.vector.tensor_tensor(
            out=stage[:, :NB * m.c1].rearrange("p (b f) -> p b f", b=NB),
            in0=h_sb[:, :NB * m.c1].rearrange("p (b f) -> p b f", b=NB),
            in1=dis[:].unsqueeze(2).broadcast_to([128, NB, m.c1]),
            op=OP.mult)
        stage_to_table(2, m.c1)

        # ------------ conv2 ------------
        T1p = bigtile(F)

        def h2_prop1(i, ps):
            nc.vector.tensor_scalar(
                out=T1p[:, ts(i, m.c1)], in0=ps,
                scalar1=negdis[:, ds(i, 1)], scalar2=None, op0=OP.mult)
            nc.vector.tensor_scalar(
                out=stage[:, ts(i, m.c1)], in0=ps,
                scalar1=negdis2[:, ds(i, 1)], scalar2=None, op0=OP.mult)

        propagate(tb[2][:], m.c1, h2_prop1)
        stage_to_table(3, m.c1)

        T2p = bigtile(F)

        def h2_prop2(i, ps):
            t = ep.tile([128, F], f32, tag="tmp", name="tmp")
            nc.vector.tensor_scalar(
                out=t[:, :m.c1], in0=ps,
                scalar1=negdisx2[:, ds(i, 1)], scalar2=None, op0=OP.mult)
            nc.vector.tensor_tensor(
                out=T2p[:, ts(i, m.c1)], in0=t[:, :m.c1],
                in1=h_sb[:, ts(i, m.c1)], op=OP.subtract)

        propagate(tb[3][:], m.c1, h2_prop2)

        # dense conv2 + final linear
        out_sb = stgp.tile([128, NB * m.out_f], bf16, tag="out_sb",
                           name="out_sb")

        def dense2_body(i):
            hp = dense3([h_sb, T1p, T2p], w2_s, m.c1, m.c2, i)
            h2 = ep.tile([128, m.c2], f32, tag="h2", name="h2")
            nc.vector.tensor_tensor(out=h2[:], in0=hp[:, :m.c2], in1=b2r_s[:],
                                    op=OP.add)
            nc.vector.tensor_scalar(out=h2[:], in0=h2[:], scalar1=0.0,
                                    scalar2=None, op0=OP.max)
            h2b = ep.tile([128, m.c2], bf16, tag="h2b", name="h2b")
            nc.vector.tensor_copy(out=h2b[:], in_=h2[:])
            tp = pp.tile([F, 128], bf16, tag="tp", name="tp")
            nc.tensor.transpose(out=tp[:m.c2, :], in_=h2b[:], identity=id_s[:])
            h2T = ep.tile([F, 128], bf16, tag="tT", name="tT")
            nc.scalar.copy(out=h2T[:m.c2, :], in_=tp[:m.c2, :])
            op_ps = pp.tile([128, m.out_f], f32, tag="stats", name="stats")
            nc.tensor.matmul(out=op_ps[:], lhsT=h2T[:m.c2, :], rhs=linwt_s[:],
                             start=True, stop=True)
            nc.vector.tensor_tensor(out=out_sb[:, ts(i, m.out_f)],
                                    in0=op_ps[:], in1=linbr_s[:], op=OP.add)

        with tc.For_i(0, NB, 2) as i:
            dense2_body(i)
            dense2_body(i + 1)

        nc.sync.dma_start(
            out=T["out"][:].rearrange("(b p) f -> p b f", p=128),
            in_=out_sb[:].rearrange("p (b f) -> p b f", b=NB))


# ---------------------------------------------------------------------------
# Entry point
# ---------------------------------------------------------------------------


def _run(inputs, n_cores=8, trace=False):
    from concourse.bass_utils import run_bass_kernel_spmd

    m, in_maps = _host_prep(n_cores=n_cores, **inputs)
    nc = _build_program(m)
    res = run_bass_kernel_spmd(nc, in_maps, core_ids=list(range(n_cores)),
                               trace=trace)
    outp = np.concatenate(
        [np.asarray(r["out"][:m.RPC], dtype=np.float32)
         for r in res.results], axis=0)
    return outp, res


def kernel(**inputs):
    out, _ = _run(inputs, n_cores=8, trace=False)
    return out
